# revision 4
# baseline (speedup 1.0000x reference)
"""Trainium2 Bass kernel for ExllamaLinear (int4 group-quantized 4096x4096 linear).

out[b,s,o] = x @ W + bias,  W[i,o] = (nib4[i,o] - z[g(i),o]) * s[g(i),o]

Strategy (8 NeuronCores, column tensor-parallel):
  - Each core owns OUT/8 = 512 output columns. Its W shard is dequantized
    ONCE at kernel start into a resident SBUF tile w3 [128, 32, 512] (f16),
    so the steady state runs with an otherwise-idle DVE/ScalarE — the PE
    matmul stream is the only real work and suffers no SBUF/power contention
    (this alone moved the matmul issue cadence from ~228 ns to its 215.8 ns
    hardware floor).
  - x is plane-permuted + transposed ONCE on the host into the exact SBUF
    layout [128, j, 8192] (i = (bb*128+p)*8+k for nibble plane k, row block
    bb, j = 4k+bb) and replicated to all cores; each core streams it in 16
    m-chunks of 512 tokens, triple-buffered on the sync DMA queue.
  - Host pre-expands scales and zero*scale to [128, 4, 512]; the group index
    8*bb + p//16 is plane-independent, so one resident tile pair feeds all 8
    planes. Device dequant per plane: shift/and (DVE), int->f16 copy
    (ScalarE), mult, subtract (DVE).
  - fp8 fraction: nibble plane 7 (4 of 32 k-tiles) runs in e4m3 with
    perf_mode=DoubleRow (two k-tiles per PE pass at ~2x rate): its weights
    are cast f16->e4m3 on device, its x slice ships as e4m3 from the host.
    Measured end-to-end rel err 0.0158 vs the 2e-2 gate, bit-identical to
    the ml_dtypes simulation of the same split.
  - Per m-chunk: 4 PSUM accumulators (one per 128-column o-tile), loop j
    outer / o-tile inner so chunk 0 consumes dequant output in production
    order; the last chunk runs o-tile outer so evictions overlap the final
    accumulations. Eviction adds the per-partition bias on ScalarE
    (activation Identity with bias AP); results DMA out via the scalar queue.
  - Host reassembles: out core-major [128, 4, 16, 512] -> [8192, 4096].
"""
import numpy as np

import concourse.bass as bass
import concourse.tile as tile
from concourse import bacc, mybir
from concourse.bass_utils import run_bass_kernel_spmd

N_CORES = 8
B, S, IN, OUT = 4, 2048, 4096, 4096
GROUP_SIZE = 128
M_TOT = B * S                  # 8192 tokens
OCC = OUT // N_CORES           # 512 output columns per core
NOT = OCC // 128               # 4 o-tiles per core
NJ = IN // 128                 # 32 contraction k-tiles
NB = 4                         # row blocks per nibble plane (IN/8/128)
MC = 512                       # tokens per m-chunk
NMC = M_TOT // MC              # 16 m-chunks
NJF = NJ - NB                  # 28 f16 k-tiles; nibble plane 7 runs fp8

f16 = mybir.dt.float16
f8 = mybir.dt.float8e4
f32 = mybir.dt.float32
i32 = mybir.dt.int32
op = mybir.AluOpType
AF = mybir.ActivationFunctionType


def build_nc():
    nc = bacc.Bacc("TRN2", target_bir_lowering=False, debug=False)

    xt_d = nc.dram_tensor("xt", [128, NJF, M_TOT], f16, kind="ExternalInput")
    xt8_d = nc.dram_tensor("xt8", [128, NB, M_TOT], f8, kind="ExternalInput")
    qw_d = nc.dram_tensor("qw", [128, NB, OCC], i32, kind="ExternalInput")
    sexp_d = nc.dram_tensor("sexp", [128, NB, OCC], f16, kind="ExternalInput")
    zsexp_d = nc.dram_tensor("zsexp", [128, NB, OCC], f16, kind="ExternalInput")
    bias_d = nc.dram_tensor("biascol", [128, NOT], f16, kind="ExternalInput")
    out_d = nc.dram_tensor("out", [128, NOT, NMC, MC], f16, kind="ExternalOutput")

    with tile.TileContext(nc) as tc:
        with (
            tc.tile_pool(name="persist", bufs=1) as pp,
            tc.tile_pool(name="work", bufs=1) as wp,
            tc.tile_pool(name="psum", bufs=1, space="PSUM") as psp,
        ):
            # ---- one-time weight-shard load + dequant ----------------------
            # the expanded scale / zero*scale tiles are identical for all 8
            # nibble planes (group index 8*bb + p//16 doesn't depend on the
            # plane), so 2 MB of one-time DMA feeds the whole dequant
            qw = pp.tile([128, NB, OCC], i32)
            nc.gpsimd.dma_start(qw[:, :NB // 2, :], qw_d[:, :NB // 2, :])
            nc.scalar.dma_start(qw[:, NB // 2:, :], qw_d[:, NB // 2:, :])
            sec = pp.tile([128, NB, OCC], f16)
            nc.gpsimd.dma_start(sec[:], sexp_d[:])
            zsc = pp.tile([128, NB, OCC], f16)
            nc.scalar.dma_start(zsc[:], zsexp_d[:])
            biasc = pp.tile([128, NOT], f16)
            nc.gpsimd.dma_start(biasc[:], bias_d[:])

            w3 = pp.tile([128, NJ, OCC], f16)
            for k in range(8):
                jsl = slice(k * NB, (k + 1) * NB)
                nib4 = wp.tile([128, NB, OCC], i32, tag="nib4", bufs=2,
                               name=f"nib4_{k}")
                nc.vector.tensor_scalar(
                    out=nib4[:], in0=qw[:], scalar1=4 * k, scalar2=0xF,
                    op0=op.logical_shift_right, op1=op.bitwise_and)
                nibf = wp.tile([128, NB, OCC], f16, tag="nibf", bufs=2,
                               name=f"nibf{k}")
                nc.scalar.copy(nibf[:], nib4[:])
                wsl = w3[:, jsl, :]
                nc.vector.tensor_tensor(wsl, nibf[:], sec[:], op.mult)
                nc.vector.tensor_tensor(wsl, wsl, zsc[:], op.subtract)
            # plane 7 runs in fp8 via DoubleRow: cast its f16 weights to e4m3
            w8 = pp.tile([128, NB, OCC], f8)
            nc.scalar.copy(w8[:], w3[:, NJF:NJ, :])

            # ---- token stream ----------------------------------------------
            # chunk DMAs ride the sync queue; chunk 0 lands in quarters so
            # the first matmuls start as soon as plane-0 dequant finishes
            xtc = [None] * NMC

            def issue_chunk(mc, parts):
                t = wp.tile([128, NJF, MC], f16, tag="xtc", bufs=3,
                            name=f"xtc{mc}")
                t8 = wp.tile([128, NB, MC], f8, tag="xtc8", bufs=3,
                             name=f"xtc8_{mc}")
                msl = slice(mc * MC, (mc + 1) * MC)
                step = NJF // parts
                for q in range(parts):
                    jsl = slice(q * step, (q + 1) * step)
                    nc.sync.dma_start(t[:, jsl, :], xt_d[:, jsl, msl])
                nc.sync.dma_start(t8[:], xt8_d[:, :, msl])
                return t, t8

            xtc[0] = issue_chunk(0, 4)
            xtc[1] = issue_chunk(1, 2)

            def evict(ps, ot, mc):
                ot_t = wp.tile([128, MC], f16, tag="ot", bufs=6, name="ot")
                nc.scalar.activation(ot_t[:], ps[:], AF.Identity,
                                     bias=biasc[:, ot:ot + 1], scale=1.0)
                nc.scalar.dma_start(out_d[:, ot, mc, :], ot_t[:])

            for mc in range(NMC):
                if mc + 2 < NMC:
                    xtc[mc + 2] = issue_chunk(mc + 2, 2)
                xt, xt8 = xtc[mc]
                pss = [psp.tile([128, MC], f32, tag="ps", bufs=8,
                                name=f"ps{mc}_{ot}") for ot in range(NOT)]
                DR = mybir.MatmulPerfMode.DoubleRow

                def acc(ps, ot, j):
                    nc.tensor.matmul(
                        ps[:], w3[:, j, ot * 128:(ot + 1) * 128],
                        xt[:, j, :], start=(j == 0), stop=False)

                def acc8(ps, ot, q):
                    nc.tensor.matmul(
                        ps[:], w8[:, 2 * q:2 * q + 2, ot * 128:(ot + 1) * 128],
                        xt8[:, 2 * q:2 * q + 2, :], start=False, stop=(q == 1),
                        perf_mode=DR)

                if mc < NMC - 1:
                    # j outer: consumes dequant output in production order
                    # (matters for chunk 0) and keeps 4 accumulators hot
                    for j in range(NJF):
                        for ot, ps in enumerate(pss):
                            acc(ps, ot, j)
                    for q in range(2):
                        for ot, ps in enumerate(pss):
                            acc8(ps, ot, q)
                    for ot, ps in enumerate(pss):
                        evict(ps, ot, mc)
                else:
                    # last chunk: o-tile outer so evictions overlap the
                    # remaining accumulations instead of trailing the kernel
                    for ot, ps in enumerate(pss):
                        for j in range(NJF):
                            acc(ps, ot, j)
                        for q in range(2):
                            acc8(ps, ot, q)
                        evict(ps, ot, mc)

    nc.compile()
    return nc


def shard_inputs(x, qweight, qzeros, scales, bias):
    """Host-side relayout into the exact per-core SBUF shapes."""
    x2 = np.asarray(x, dtype=np.float16).reshape(M_TOT, IN)
    qweight = np.asarray(qweight, dtype=np.int32)
    qzeros = np.asarray(qzeros, dtype=np.int32)
    scales = np.asarray(scales, dtype=np.float16)
    bias = np.asarray(bias, dtype=np.float16)

    # x -> [p, j=k*NB+bb, m] with i = (bb*128+p)*8 + k; plane 7 ships fp8
    import ml_dtypes
    xp = x2.reshape(M_TOT, NB, 128, 8).transpose(2, 3, 1, 0)  # [p, k, bb, m]
    xt = np.ascontiguousarray(xp[:, :7].reshape(128, NJF, M_TOT))
    xt8 = np.ascontiguousarray(xp[:, 7]).astype(ml_dtypes.float8_e4m3fn)

    # unpack zeros: z[gg, o]
    sh = (np.arange(8, dtype=np.int32) * 4)
    z = ((qzeros[:, :, None] >> sh[None, None, :]) & 0xF).reshape(
        qzeros.shape[0], -1)
    zs = (z.astype(np.float16) * scales).astype(np.float16)  # [G, OUT]

    # group index per (p, bb): g = 8*bb + p//16 (plane-independent)
    g2 = 8 * np.arange(NB)[None, :] + np.arange(128)[:, None] // 16  # [128,NB]

    in_maps = []
    for c in range(N_CORES):
        osl = slice(c * OCC, (c + 1) * OCC)
        qw_core = np.ascontiguousarray(
            qweight[:, osl].reshape(NB, 128, OCC).transpose(1, 0, 2))
        in_maps.append({
            "xt": xt, "xt8": xt8,
            "qw": qw_core,
            "sexp": np.ascontiguousarray(scales[:, osl][g2]),  # [128,NB,OCC]
            "zsexp": np.ascontiguousarray(zs[:, osl][g2]),
            "biascol": np.ascontiguousarray(
                bias[osl].reshape(NOT, 128).T),
        })
    return in_maps


def assemble(results):
    """results[c]["out"] is [128, NOT, NMC, MC] -> full [M_TOT, OUT] f16."""
    cols = []
    for c in range(N_CORES):
        o = results[c]["out"]                    # [128, 4, 16, 512]
        cols.append(o.transpose(2, 3, 1, 0).reshape(M_TOT, OCC))
    return np.concatenate(cols, axis=1)


_NC_CACHE = {}


def kernel(x, qweight, qzeros, scales, bias):
    if "nc" not in _NC_CACHE:
        _NC_CACHE["nc"] = build_nc()
    nc = _NC_CACHE["nc"]
    in_maps = shard_inputs(x, qweight, qzeros, scales, bias)
    res = run_bass_kernel_spmd(nc, in_maps, list(range(N_CORES)))
    out = assemble(res.results)
    return out.reshape(B, S, OUT).astype(np.float16)
